# revision 27
# baseline (speedup 1.0000x reference)
"""Trainium2 Bass kernel for the AdreQwen2 MoE-LoRA SwiGLU MLP.

Problem (hardcoded): B=4, S=2048, H=2048, I=5504, E=8 experts, top-2
per-batch binary gating, rank-16 LoRA adapters on gate/up/down, scale 2.0.

Distribution: token-parallel across 8 NeuronCores (1024 tokens each; each
core's tokens belong to exactly one batch, so its 2 active experts are
fixed). The host pre-selects the top-2 experts per batch and folds the
LoRA adapters into the dense weights exactly (binary gates make this pure
linear algebra): W_eff = W + 2.0 * (A_e0|A_e1 @ B_e0|B_e1)^T. The device
kernel is then a pure dense SwiGLU MLP in bf16 (same 1 col/cycle PE rate
as f32r on TRN2, half the DMA + SBUF; rel err ~3.4e-3 vs the 2e-2 gate).
No collectives: outputs are disjoint token slices, concatenated on host.

Device kernel design (per core, PE floor 4128 matmuls x ~215.8 ns = 891 us;
the extra 2.5 ns/MM over 512cols/2.4GHz is the NX dispatch floor — bf16
PSUM outputs that would allow 1024-col matmuls are TRN3-only):
 - Fused phases: h = silu(Wg x) * (Wu x) stays RESIDENT in SBUF as bf16
   (86 KB/partition), never spilled to DRAM; phase 2 (Wd h) accumulates
   all 43 K-tiles of an output tile in a single PSUM bank (no SBUF adds).
 - x lives in one resident SBUF tile; weights double-buffer through small
   pools (bf16 streams: Wg+Wu 45 MB, Wd 22.5 MB per core).
 - Head is DMA-bytes-bound (~390 GB/s from ~8.6 us): 30 N=256 warm-up
   matmuls on a small memset tile keep the PE busy (and the HAM clock
   gate warm at 2.4 GHz) until the head stream is ahead (~13 us), with 4 single-MM pads woven
   into the first real group at the x-pacing gap spots; real matmuls
   then run back-to-back with no HAM re-throttle. Shorter
   warm-ups measurably convert dummy time into DMA-wait gaps and risk
   HAM oscillation (re-throttle costs ~2x on every matmul in a 3.4+ us
   idle window).
 - mi 0/1 run chunk-0-first across both mi groups so x-chunk1 (last head
   delivery, ~24 us) is not needed until MM #64 (~27 us); triggers are
   issued in exact consumption order at ko-pair granularity (each
   dma_start costs ~0.63 us serially on the Sync queue).
 - Steady state: consecutive matmuls share their stationary weight slice
   (both token chunks per ko); LDWEIGHTS (97 ns) hides under the 213 ns
   matmul stream on the second SBUF read port.
 - Tail: the last output tile's second token-chunk accumulates in two
   256-col PSUM groups in SEPARATE banks so Scalar and Vector drain them
   concurrently, with the two stores on different HWDGE queues (sync,
   scalar). Post-last-matmul chain ~4.9 us (copy 0.43 + trigger 0.6 +
   HBM receipt ~1.5 + fixed epilogue ~2.4).
Measured: ~911 us HW exec (prior session baseline 913.6 us; f32r spill
version 972 us), ~98.9% of the per-instruction PE floor.
"""

import sys
import types

import numpy as np

# ---- problem constants (must match setup_inputs) ----
B, S, H, I, E, R = 4, 2048, 2048, 5504, 8, 16
TOP_K = 2
LORA_SCALE = 32.0 / 16.0

P = 128
KH = H // P          # 16 K-tiles over H
KI = I // P          # 43 K-tiles / M-tiles over I
MH = H // P          # 16 M-tiles over H (phase 2 output)
N_CORES = 8
T = B * S            # 8192 tokens
T_CORE = T // N_CORES  # 1024 tokens per core
TCH = 512            # token chunk (matmul moving dim; PSUM bank = 512 f32)
NCHUNK = T_CORE // TCH  # 2

_CACHE: dict = {}


def install_ntff_hook():
    """The antenv stub in this image lacks axon_hooks; reconstruct it so
    run_bass_kernel_spmd(trace=True) can capture NTFF profiles."""
    if "antenv.axon_hooks" in sys.modules:
        return
    try:
        mod = types.ModuleType("antenv.axon_hooks")
        mod._hook = None
        mod.set_axon_ntff_profile_hook = lambda h: setattr(mod, "_hook", h)
        mod.get_axon_ntff_profile_hook = lambda: mod._hook
        sys.modules["antenv.axon_hooks"] = mod
        from trn_agent_boot.trn_boot import _ntff_profile_via_ctypes

        mod.set_axon_ntff_profile_hook(
            _ntff_profile_via_ctypes("/opt/axon/libaxon_pjrt.so")
        )
    except Exception:
        sys.modules.pop("antenv.axon_hooks", None)


def _build_nc():
    import concourse.bacc as bacc
    import concourse.mybir as mybir
    import concourse.tile as tile
    from concourse.bass import ts

    f32 = mybir.dt.float32
    bf16 = mybir.dt.bfloat16
    silu_fn = mybir.ActivationFunctionType.Silu

    nc = bacc.Bacc()

    x_t = nc.declare_dram_parameter("x_t", [P, KH, T_CORE], bf16, isOutput=False)
    wg_t = nc.declare_dram_parameter("wg_t", [KI, P, KH, P], bf16, isOutput=False)
    wu_t = nc.declare_dram_parameter("wu_t", [KI, P, KH, P], bf16, isOutput=False)
    wd_t = nc.declare_dram_parameter("wd_t", [MH, P, KI, P], bf16, isOutput=False)
    outT = nc.declare_dram_parameter("outT", [H, T_CORE], f32, isOutput=True)

    with (
        tile.TileContext(nc) as tc,
        tc.tile_pool(name="xp", bufs=1) as xp,
        tc.tile_pool(name="hp", bufs=1) as hp,
        tc.tile_pool(name="wmp", bufs=1) as wmp,
        tc.tile_pool(name="wgp", bufs=3) as wgp,
        tc.tile_pool(name="wup", bufs=3) as wup,
        tc.tile_pool(name="wdp", bufs=3) as wdp,
        tc.tile_pool(name="work", bufs=3) as work,
        tc.tile_pool(name="outp", bufs=4) as outp,
        tc.tile_pool(name="psg", bufs=2, space="PSUM") as psg,
        tc.tile_pool(name="psup", bufs=2, space="PSUM") as psup,
        tc.tile_pool(name="pso", bufs=4, space="PSUM") as pso,
    ):
        def load_w(pool, tag, src, mi, nko=KH, nsplit=4):
            w_sb = pool.tile([P, nko, P], bf16, tag=tag, name=f"{tag}_{mi}")
            bounds = [nko * q // nsplit for q in range(nsplit + 1)]
            for a, b in zip(bounds, bounds[1:]):
                nc.sync.dma_start(w_sb[:, a:b, :], src[mi][:, a:b, :])
            return w_sb

        # PE warm-up: small-N matmuls on a tiny zero tile, issued before any
        # real work so the HAM clock gate reaches 2.4 GHz while the first
        # x/weight DMAs are still in flight. The memset is small ([128,256]
        # bf16) so the warm-up starts as soon as the Vector engine is up
        # (~7.3 us) rather than gating on a 640-col memset. PSUM result is
        # never read.
        warm = wmp.tile([P, 3 * P], bf16, tag="warm", name="warm")
        nc.vector.memset(warm[:], 0.0)
        pw = psg.tile([P, TCH], f32, tag="g", name="pg_warm")
        # N=256 dummies at the cold clock (213 ns each) give the HAM monitor
        # its ~3.4 us of sustained busy (plus window-phase margin) and keep
        # the PE occupied until the head DMA stream is comfortably ahead
        # (~13 us). Starting real matmuls earlier just converts dummy time
        # into DMA-wait gaps (measured: 18 dummies -> 5.8 us of gaps, net
        # worse).
        NWARM = 30
        for j in range(NWARM):
            nc.tensor.matmul(
                pw[:, : 2 * P],
                warm[:, 2 * P : 3 * P],
                warm[:, : 2 * P],
                start=(j == 0),
                stop=(j == NWARM - 1),
            )

        # x lives in one resident SBUF tile. Trigger order = exact consumption
        # order of mi=0's chunk-serial matmul groups (wg-c0, wu-c0, wg-c1,
        # wu-c1): wg0 half, x-c0 ko-pairs interleaved, wu0 halves mid-stream,
        # then x-c1 quads. This keeps the PE DMA-paced (not quarter-granular
        # stalled) through the bandwidth-bound head.
        x_sb = xp.tile([P, KH, T_CORE], bf16, tag="x", name="x_sb")
        wg0 = wgp.tile([P, KH, P], bf16, tag="wg", name="wg_0")
        wu0 = wup.tile([P, KH, P], bf16, tag="wu", name="wu_0")
        nc.sync.dma_start(wg0[:, 0:8, :], wg_t[0][:, 0:8, :])
        nc.sync.dma_start(x_sb[:, 0:2, ts(0, TCH)], x_t[:, 0:2, ts(0, TCH)])
        nc.sync.dma_start(x_sb[:, 2:4, ts(0, TCH)], x_t[:, 2:4, ts(0, TCH)])
        nc.sync.dma_start(wg0[:, 8:16, :], wg_t[0][:, 8:16, :])
        nc.sync.dma_start(x_sb[:, 4:6, ts(0, TCH)], x_t[:, 4:6, ts(0, TCH)])
        nc.sync.dma_start(x_sb[:, 6:8, ts(0, TCH)], x_t[:, 6:8, ts(0, TCH)])
        nc.sync.dma_start(wu0[:, 0:8, :], wu_t[0][:, 0:8, :])
        nc.sync.dma_start(x_sb[:, 8:10, ts(0, TCH)], x_t[:, 8:10, ts(0, TCH)])
        nc.sync.dma_start(x_sb[:, 10:12, ts(0, TCH)], x_t[:, 10:12, ts(0, TCH)])
        nc.sync.dma_start(wu0[:, 8:16, :], wu_t[0][:, 8:16, :])
        nc.sync.dma_start(x_sb[:, 12:14, ts(0, TCH)], x_t[:, 12:14, ts(0, TCH)])
        nc.sync.dma_start(x_sb[:, 14:16, ts(0, TCH)], x_t[:, 14:16, ts(0, TCH)])
        # mi-1 weights stream BEFORE x-c1: the chunk-0-first schedule below
        # consumes (wg0,wu0,wg1,wu1) on chunk 0 through MM #63 and only needs
        # x-c1 from MM #64 (~27 us), after x-c1's ~24 us delivery.
        wg1 = load_w(wgp, "wg", wg_t, 1, nsplit=2)
        wu1 = load_w(wup, "wu", wu_t, 1, nsplit=2)
        for q in range(4):
            ks = slice(4 * q, 4 * q + 4)
            nc.sync.dma_start(x_sb[:, ks, ts(1, TCH)], x_t[:, ks, ts(1, TCH)])
        x_tiles = [[x_sb[:, ko, ts(n, TCH)] for ko in range(KH)] for n in range(NCHUNK)]

        # ---------------- phase 1: h = silu(Wg x) * (Wu x), resident ----------------
        # Each weight slice feeds both token chunks back-to-back (two PSUM
        # accumulation groups interleaved per ko) so consecutive matmuls share
        # their stationary operand — halves LDWEIGHTS work if codegen dedupes.
        h_tiles = {}

        def p1_finish(mi, n, pg_t, pup_t):
            sil = work.tile([P, TCH], f32, tag="sil", name=f"sil_{mi}_{n}")
            nc.scalar.activation(sil[:], pg_t[:], silu_fn)
            ht = hp.tile([P, TCH], bf16, tag=f"h{n}_{mi}", name=f"h_{n}_{mi}")
            nc.vector.tensor_mul(out=ht[:], in0=sil[:], in1=pup_t[:])
            h_tiles[(n, mi)] = ht

        # mi 0/1 run chunk-0-first across BOTH mi groups (mi0-c0, mi1-c0,
        # mi0-c1, mi1-c1): the first 64 matmuls touch only x-c0 + four weight
        # tiles, so x-c1 (delivered ~24 us) is not needed until MM #64
        # (~27 us). Groups close strictly serially, so psg/psup bufs=2 still
        # suffice.
        for n in range(NCHUNK):
            for mi, wg_sb, wu_sb in ((0, wg0, wu0), (1, wg1, wu1)):
                pg_t = psg.tile([P, TCH], f32, tag="g", name=f"pg_{mi}_{n}")
                pup_t = psup.tile([P, TCH], f32, tag="up", name=f"pup_{mi}_{n}")
                for mat, pt in ((wg_sb, pg_t), (wu_sb, pup_t)):
                    for ko in range(KH):
                        nc.tensor.matmul(
                            pt[:],
                            mat[:, ko, :],
                            x_tiles[n][ko],
                            start=(ko == 0),
                            stop=(ko == KH - 1),
                        )
                        if n == 0 and mi == 0 and mat is wg_sb and ko in (5, 7, 9, 11):
                            # single-MM pads at the observed x-pacing gap
                            # spots: they run for free while the PE would
                            # wait on the next x ko-pair, keeping the HAM
                            # activity window busy (an idle window here
                            # re-throttles the clock to 1.2 GHz for ~2 us)
                            nc.tensor.matmul(
                                pw[:, : 2 * P],
                                warm[:, 2 * P : 3 * P],
                                warm[:, : 2 * P],
                                start=True,
                                stop=True,
                            )
                p1_finish(mi, n, pg_t, pup_t)

        for mi in range(2, KI):
            # steady state needs only 4 triggers per mi (halves)
            wg_sb = load_w(wgp, "wg", wg_t, mi, nsplit=2)
            wu_sb = load_w(wup, "wu", wu_t, mi, nsplit=2)
            pg = [psg.tile([P, TCH], f32, tag="g", name=f"pg_{mi}_{n}") for n in range(NCHUNK)]
            pup = [psup.tile([P, TCH], f32, tag="up", name=f"pup_{mi}_{n}") for n in range(NCHUNK)]
            # weight-reuse interleaving: both chunks per stationary slice
            for mat, psl in ((wg_sb, pg), (wu_sb, pup)):
                for ko in range(KH):
                    for n in range(NCHUNK):
                        nc.tensor.matmul(
                            psl[n][:],
                            mat[:, ko, :],
                            x_tiles[n][ko],
                            start=(ko == 0),
                            stop=(ko == KH - 1),
                        )
            for n in range(NCHUNK):
                p1_finish(mi, n, pg[n], pup[n])

        # ---------------- phase 2: outT = Wd h (PSUM-accumulated) ----------------
        HW = TCH // 2
        for mh in range(MH):
            wd_sb = load_w(wdp, "wd", wd_t, mh, nko=KI)
            if mh < MH - 1:  # weight-reuse interleaving
                po = [pso.tile([P, TCH], f32, tag="o", name=f"po_{mh}_{n}") for n in range(NCHUNK)]
                for ki in range(KI):
                    for n in range(NCHUNK):
                        nc.tensor.matmul(
                            po[n][:],
                            wd_sb[:, ki, :],
                            h_tiles[(n, ki)][:],
                            start=(ki == 0),
                            stop=(ki == KI - 1),
                        )
                for n in range(NCHUNK):
                    ob = outp.tile([P, TCH], f32, tag="ob", name=f"ob_{mh}_{n}")
                    nc.scalar.copy(ob[:], po[n][:])
                    nc.sync.dma_start(outT[ts(mh, P), ts(n, TCH)], ob[:])
            else:
                # Last mh: n=0 runs as one serial group first (its copy+store
                # hide under n=1's matmuls). n=1 is split into two 256-col
                # groups in SEPARATE PSUM banks so the Scalar and Vector
                # engines can drain them in parallel (no same-bank access
                # hazard), with the two stores on different HWDGE queues.
                # This shortens the post-last-matmul chain to one 256-col
                # copy + one store trigger.
                po0 = pso.tile([P, TCH], f32, tag="o", name=f"po_{mh}_0")
                for ki in range(KI):
                    nc.tensor.matmul(
                        po0[:],
                        wd_sb[:, ki, :],
                        h_tiles[(0, ki)][:],
                        start=(ki == 0),
                        stop=(ki == KI - 1),
                    )
                ob0 = outp.tile([P, TCH], f32, tag="ob", name=f"ob_{mh}_0")
                nc.scalar.copy(ob0[:], po0[:])
                nc.sync.dma_start(outT[ts(mh, P), ts(0, TCH)], ob0[:])
                # symmetric split: balances the two copy+store+receipt chains
                # (scalar+sync vs vector+scalar), minimizing the slower one
                WA, WB = 2 * P, 2 * P
                poa = pso.tile([P, TCH], f32, tag="o", name=f"po_{mh}_1a")
                pob = pso.tile([P, TCH], f32, tag="o", name=f"po_{mh}_1b")
                for ki in range(KI):
                    nc.tensor.matmul(
                        poa[:, :WA],
                        wd_sb[:, ki, :],
                        h_tiles[(1, ki)][:, :WA],
                        start=(ki == 0),
                        stop=(ki == KI - 1),
                    )
                    nc.tensor.matmul(
                        pob[:, :WB],
                        wd_sb[:, ki, :],
                        h_tiles[(1, ki)][:, WA : WA + WB],
                        start=(ki == 0),
                        stop=(ki == KI - 1),
                    )
                oba = outp.tile([P, WA], f32, tag="obq", name=f"ob_{mh}_1a")
                obb = outp.tile([P, WB], f32, tag="obr", name=f"ob_{mh}_1b")
                nc.scalar.copy(oba[:], poa[:, :WA])
                nc.sync.dma_start(outT[ts(mh, P), TCH : TCH + WA], oba[:])
                nc.vector.tensor_scalar_add(obb[:], pob[:, :WB], 0.0)
                nc.scalar.dma_start(outT[ts(mh, P), TCH + WA : 2 * TCH], obb[:])

    nc.finalize()
    return nc


def _get_nc():
    if "nc" not in _CACHE:
        _CACHE["nc"] = _build_nc()
    return _CACHE["nc"]


def _tile_kxm(w, n_m, n_k):
    """(M, K) row-major -> (n_m, P, n_k, P) with [mi, p, ko, m] = w[128mi+m, 128ko+p]."""
    return np.ascontiguousarray(w.reshape(n_m, P, n_k, P).transpose(0, 3, 2, 1))


def _prep_inputs(x, gate_values, Wg, Ag, Bg, Wu, Au, Bu, Wd, Ad, Bd):
    """Host-side expert selection, LoRA folding, sharding, and layout prep."""
    import ml_dtypes

    f32 = np.float32
    bf16 = ml_dtypes.bfloat16
    c = np.ascontiguousarray

    xf = np.asarray(x, f32).reshape(T, H)
    gv = np.asarray(gate_values, f32)
    idx = np.argsort(-gv, axis=1)[:, :TOP_K]  # (B, 2) top-2 experts per batch

    Wg_, Wu_, Wd_ = np.asarray(Wg, f32), np.asarray(Wu, f32), np.asarray(Wd, f32)
    Ag_, Bg_ = np.asarray(Ag, f32), np.asarray(Bg, f32)
    Au_, Bu_ = np.asarray(Au, f32), np.asarray(Bu, f32)
    Ad_, Bd_ = np.asarray(Ad, f32), np.asarray(Bd, f32)

    per_batch = []
    for b in range(B):
        es = [int(idx[b, 0]), int(idx[b, 1])]
        # exact LoRA fold: binary top-2 gates => W_eff = W + s * (A_cat @ B_cat)^T
        ag = np.concatenate([Ag_[e] for e in es], axis=1)  # (H, 2R)
        bg = np.concatenate([Bg_[e] for e in es], axis=0)  # (2R, I)
        au = np.concatenate([Au_[e] for e in es], axis=1)
        bu = np.concatenate([Bu_[e] for e in es], axis=0)
        ad = np.concatenate([Ad_[e] for e in es], axis=1)  # (I, 2R)
        bd = np.concatenate([Bd_[e] for e in es], axis=0)  # (2R, H)
        wg_eff = Wg_ + LORA_SCALE * (ag @ bg).T            # (I, H)
        wu_eff = Wu_ + LORA_SCALE * (au @ bu).T            # (I, H)
        wd_eff = Wd_ + LORA_SCALE * (ad @ bd).T            # (H, I)
        per_batch.append(
            (
                _tile_kxm(wg_eff, KI, KH).astype(bf16),
                _tile_kxm(wu_eff, KI, KH).astype(bf16),
                _tile_kxm(wd_eff, MH, KI).astype(bf16),
            )
        )

    in_maps = []
    for core in range(N_CORES):
        b = core * T_CORE // S  # batch this core's tokens belong to
        xc = xf[core * T_CORE : (core + 1) * T_CORE]               # (1024, H)
        x_tl = c(xc.T.reshape(KH, P, T_CORE).transpose(1, 0, 2)).astype(bf16)
        wg_tb, wu_tb, wd_tb = per_batch[b]
        in_maps.append({"x_t": x_tl, "wg_t": wg_tb, "wu_t": wu_tb, "wd_t": wd_tb})
    return in_maps


def _run(inputs, trace=False):
    from concourse.bass_utils import run_bass_kernel_spmd

    if trace:
        install_ntff_hook()
    nc = _get_nc()
    in_maps = _prep_inputs(**inputs)
    res = None
    last_err = None
    for attempt in range(3):  # transient NRT/axon execution errors are retriable
        try:
            res = run_bass_kernel_spmd(
                nc, in_maps, core_ids=list(range(N_CORES)), trace=trace
            )
            break
        except Exception as e:
            last_err = e
    if res is None:
        raise last_err
    outs = [res.results[c]["outT"] for c in range(N_CORES)]  # (H, 1024) each
    full = np.concatenate([o.T for o in outs], axis=0)       # (T, H)
    return full.reshape(B, S, H).astype(np.float32), res


def kernel(**inputs):
    out, _ = _run(inputs, trace=False)
    return out

